# revision 17
# baseline (speedup 1.0000x reference)
"""CRF constituency marginals (inside-outside, scaled exp space) on 8 TRN2 cores.

v4: inside and outside passes run CONCURRENTLY (outside width u needs only
inside diagonals <= 255-u; with a 3-step lag both 255-step chains overlap,
halving serial depth). Bulk R-alignment uses bf16 hi-only chart values
(~1e-2 rel err, under the 2e-2 gate). Outside R1/R2 build 1-ahead (2 banks
each); per-step U/GU maintenance stages through a single shared PSUM bank
whose access order is serialized by overlapping access patterns (PE writes
-> scalar reads -> next-step PE writes via WAR on the same columns).

PSUM banks (8): psI x3 inside R | psR x2 outside R1 | psR2 x2 outside R2
 | psM x1 maintenance staging.
Charts are col-grouped: logical row r -> (partition r%128, col-block r//128).
"""
import sys, os
import numpy as np

for p in ("/opt/trn_rl_repo", "/root/.axon_site/_ro/trn_rl_repo"):
    if os.path.isdir(p) and p not in sys.path:
        sys.path.append(p)

import concourse.bass as bass
import concourse.bacc as bacc
import concourse.mybir as mybir
import concourse.tile as tile
from concourse.bass_utils import run_bass_kernel_spmd
import bass_rust

S = 256
C = 1.9
dt = mybir.dt.float32
bf = mybir.dt.bfloat16
MULT = mybir.AluOpType.mult
SUB = mybir.AluOpType.subtract
ADD = mybir.AluOpType.add
Exp = mybir.ActivationFunctionType.Exp

_cache = {}


def _ap(t_ap, offset, dims):
    return bass_rust.AP(tensor=t_ap.tensor, offset=int(offset),
                        ap=[[int(a), int(b)] for a, b in dims])


def build_nc():
    nc = bacc.Bacc("TRN2", target_bir_lowering=False)
    sc = nc.dram_tensor("scores", [S, S], dt, kind="ExternalInput")
    wu_d = nc.dram_tensor("wu", [128, 256], bf, kind="ExternalInput")
    wd_d = nc.dram_tensor("wd", [128, 384], bf, kind="ExternalInput")
    out = nc.dram_tensor("out", [S, S], dt, kind="ExternalOutput")
    DEBUG = bool(int(os.environ.get("K3_DEBUG", "0")))
    if DEBUG:
        tdump = nc.dram_tensor("tdump", [128, 512], dt, kind="ExternalOutput")
        gdump = nc.dram_tensor("gdump", [128, 1024], dt, kind="ExternalOutput")

    with tile.TileContext(nc) as tc:
        with tc.tile_pool(name="pers", bufs=1) as pool, \
             tc.tile_pool(name="vtp", bufs=4) as vtp, \
             tc.tile_pool(name="psI", bufs=2, space="PSUM") as psI, \
             tc.tile_pool(name="psR", bufs=2, space="PSUM") as psR, \
             tc.tile_pool(name="psR2", bufs=2, space="PSUM") as psR2, \
             tc.tile_pool(name="psM", bufs=1, space="PSUM") as psM:
            T = pool.tile([128, 512], dt, tag="T")       # inside chart by-start [g0|g1]
            GTU = pool.tile([128, 1024], dt, tag="GTU")  # [GTg0|U32g0|GTg1|U32g1]
            UB = pool.tile([128, 1024], bf, tag="UB")    # U hi [g0|g1|dump|dump]
            TGB = pool.tile([128, 2048], bf, tag="TGB")  # [Tg0hi|Tg1hi|-|-|GUg0hi|GUg1hi|-|-]
            E = pool.tile([128, 512], dt, tag="E")
            En = pool.tile([128, 512], dt, tag="En")
            SCR = pool.tile([128, 512], dt, tag="SCR")
            WU = pool.tile([128, 256], bf, tag="WU")
            WD = pool.tile([128, 384], bf, tag="WD")
            ONES = pool.tile([128, 128], dt, tag="ONES")
            MISC = pool.tile([128, 8], dt, tag="MISC")
            PMI = psM.tile([128, 512], dt, tag="PMI")    # inside maint staging
            PMO = psM.tile([128, 512], dt, tag="PMO")    # outside maint staging

            SU = lambda s: WU[:, s:s + 128]              # noqa: E731  out[p]=in[p+s]
            SD = lambda s: WD[:, 128 - s:256 - s]        # noqa: E731  out[p]=in[p-s]

            for t in (T, GTU, SCR):
                nc.vector.memset(t[:], 0.0)
            nc.vector.memset(UB[:], 0.0)
            nc.vector.memset(TGB[:], 0.0)
            nc.sync.dma_start(out=WU[:], in_=wu_d[:])
            nc.sync.dma_start(out=WD[:], in_=wd_d[:])
            # skewed score load: SCR[p, g*256+x] = scores[r, r+x], r = p+128g
            nc.sync.dma_start(out=SCR[:, 0:256], in_=_ap(sc[:], 0, [[257, 128], [1, 256]]))
            nc.sync.dma_start(out=SCR[0:127, 256:512], in_=_ap(sc[:], 128 * 257, [[257, 127], [1, 256]]))
            nc.scalar.activation(E[:], SCR[:], Exp, bias=0.0, scale=1.0)
            nc.scalar.activation(En[:], SCR[:], Exp, bias=0.0, scale=-1.0)

            Rit = {}

            def getRI(w):
                if w not in Rit:
                    Rit[w] = psI.tile([128, 512], dt, tag="RI", name=f"RI{w}")
                return Rit[w]

            Rt = {}

            def getR(w):
                if w not in Rt:
                    Rt[w] = psR.tile([128, 512], dt, tag="R", name=f"R{w}")
                return Rt[w]

            R2t = {}

            def getR2(w):
                if w not in R2t:
                    R2t[w] = psR2.tile([128, 512], dt, tag="R2", name=f"R2_{w}")
                return R2t[w]

            # ---------------- INSIDE ----------------
            def tgb_hi2(w):
                return _ap(TGB[:], w, [[2048, 128], [256, 2]])

            def split_in(w):
                # hi (bf16) of T diag w -> TGB cols {w, 256+w}; lo -> vtl when needed
                src = _ap(T[:], w, [[512, 128], [256, 2]])
                nc.scalar.copy(tgb_hi2(w), src)
                if w < 16:
                    vtl = vtp.tile([128, 8], bf, tag="vtl", name=f"vtl{w}")
                    nc.vector.tensor_tensor(out=vtl[:, 0:2], in0=src, in1=tgb_hi2(w), op=SUB)
                    return vtl
                return None

            def in_chain_C(w, vtl):
                # R(w+1) col m=1 = SU_1(val_w); boundary via SD_127 when val has g1 rows
                Rn = getRI(w + 1)
                lo = (w < 16)
                hi2 = tgb_hi2(w)
                hi_g0 = _ap(TGB[:], w, [[2048, 128], [1, 1]])
                hi_g1 = _ap(TGB[:], 256 + w, [[2048, 128], [1, 1]])
                ops = []
                if w + 1 <= 127:
                    o = _ap(Rn[:], 1, [[512, 128], [256, 2], [1, 1]])
                    ops.append((o, SU(1), hi2))
                    if lo:
                        ops.append((o, SU(1), vtl[:, 0:2]))
                else:
                    o = _ap(Rn[:], 1, [[512, 128], [1, 1]])
                    ops.append((o, SU(1), hi_g0))
                    if lo:
                        ops.append((o, SU(1), vtl[:, 0:1]))
                if w < 128:
                    o0 = _ap(Rn[:], 1, [[512, 128], [1, 1]])
                    ops.append((o0, SD(127), hi_g1))
                    if lo:
                        ops.append((o0, SD(127), vtl[:, 1:2]))
                for i, (oo, ss, mm) in enumerate(ops):
                    nc.tensor.matmul(oo, ss, mm, start=(i == 0),
                                     stop=(w == 1 and i == len(ops) - 1),
                                     skip_group_check=True)

            def in_maint_mm(w):
                # U col w (shift t(w) by w) staged in PM cols {384,385}
                hi2 = tgb_hi2(w)
                hi_g0 = _ap(TGB[:], w, [[2048, 128], [1, 1]])
                pm2 = _ap(PMI[:], 384, [[512, 128], [1, 2]])
                pm1 = _ap(PMI[:], 385, [[512, 128], [1, 1]])
                pm0 = _ap(PMI[:], 384, [[512, 128], [1, 1]])
                if w < 128:
                    nc.tensor.matmul(pm2, SD(w), hi2, start=True, stop=False, skip_group_check=True)
                    nc.tensor.matmul(pm1, SU(128 - w), hi_g0, start=False, stop=True, skip_group_check=True)
                else:
                    nc.tensor.matmul(pm0, SD(w - 128), hi_g0, start=True, stop=True, skip_group_check=True)

            def in_maint_copy(w):
                # U32 <- PM (fp32), then UB <- PM (bf16) with dummy read of the
                # pm-out cols {388,389} so later PE writes to PM wait for it.
                if w < 128:
                    nc.scalar.copy(_ap(GTU[:], 256 + w, [[1024, 128], [512, 2]]),
                                   _ap(PMI[:], 384, [[512, 128], [1, 2]]))
                    nc.scalar.copy(_ap(UB[:], w, [[1024, 128], [256, 2]]),
                                   _ap(PMI[:], 384, [[512, 128], [1, 2]]))
                else:
                    nc.scalar.copy(_ap(GTU[:], 768 + w, [[1024, 128], [1, 1]]),
                                   _ap(PMI[:], 384, [[512, 128], [1, 1]]))
                    nc.scalar.copy(_ap(UB[:], 256 + w, [[1024, 128], [1, 1]]),
                                   _ap(PMI[:], 384, [[512, 128], [1, 1]]))

            def in_bulk_A(wt):
                # R(wt) cols m=2..wt-1 <- U cols u=wt-m (rev), via SU/SD bands
                n = wt - 2
                if n <= 0:
                    return
                Rn = getRI(wt)
                u0 = wt - 2  # newest U col read
                if wt <= 127:
                    o = _ap(Rn[:], 2, [[512, 128], [256, 2], [1, n]])
                    nc.tensor.matmul(o, SU(wt), _ap(UB[:], u0, [[1024, 128], [256, 2], [-1, n]]),
                                     start=False, stop=False, skip_group_check=True)
                    o0 = _ap(Rn[:], 2, [[512, 128], [1, n]])
                    nc.tensor.matmul(o0, SD(128 - wt), _ap(UB[:], 256 + u0, [[1024, 128], [-1, n]]),
                                     start=False, stop=True, skip_group_check=True)
                else:
                    o0 = _ap(Rn[:], 2, [[512, 128], [1, n]])
                    nc.tensor.matmul(o0, SU(wt - 128), _ap(UB[:], 256 + u0, [[1024, 128], [-1, n]]),
                                     start=False, stop=True, skip_group_check=True)

            def in_stt(w):
                c0 = min(128, 256 - w)
                c1 = max(0, 128 - w)
                R = getRI(w)
                nc.vector.scalar_tensor_tensor(
                    out=SCR[0:c0, 0:w - 1], in0=T[0:c0, 1:w], scalar=E[0:c0, w:w + 1],
                    in1=_ap(R[:], 1, [[512, c0], [1, w - 1]]), op0=MULT, op1=MULT,
                    accum_out=T[0:c0, w:w + 1])
                if c1 > 0:
                    nc.vector.scalar_tensor_tensor(
                        out=SCR[0:c1, 256:256 + w - 1], in0=T[0:c1, 257:256 + w],
                        scalar=E[0:c1, 256 + w:257 + w],
                        in1=_ap(R[:], 257, [[512, c1], [1, w - 1]]), op0=MULT, op1=MULT,
                        accum_out=T[0:c1, 256 + w:257 + w])
                del Rit[w]

            # ---------------- OUTSIDE ----------------
            def split_out(w):
                vt = vtp.tile([128, 8], bf, tag="vt", name=f"vto{w}")
                src = _ap(GTU[:], w, [[1024, 128], [512, 2]])
                nc.scalar.copy(vt[:, 0:2], src)
                if w >= 240:
                    nc.vector.tensor_tensor(out=vt[:, 2:4], in0=src, in1=vt[:, 0:2], op=SUB)
                return vt

            def out_chain_C(w, vt):
                # R1(w-1) col s=1 = SD_1(gval_w); g1 gets SD_1(valg1) + SU_127(valg0)
                Rn = getR(w - 1)
                lo = (w >= 240)
                o = _ap(Rn[:], 257, [[512, 128], [1, 1]])
                if lo:
                    nc.tensor.matmul(o, SD(1), vt[:, 0:1], start=True, stop=False, skip_group_check=True)
                    nc.tensor.matmul(o, SD(1), vt[:, 2:3], start=False, stop=False, skip_group_check=True)
                else:
                    nc.tensor.matmul(o, SD(1), vt[:, 0:1], start=True, stop=False, skip_group_check=True)
                if w <= 128:
                    Rn2 = getR2(w - 1)
                    o2 = _ap(Rn2[:], 257, [[512, 128], [1, 1]])
                    nc.tensor.matmul(o2, SD(1), vt[:, 1:2], start=True, stop=False, skip_group_check=True)
                    nc.tensor.matmul(o2, SU(127), vt[:, 0:1], start=False, stop=False, skip_group_check=True)

            def out_maint_mm(w, vt):
                # GU col w staged in PM cols {388,389}
                pm2 = _ap(PMO[:], 384, [[512, 128], [1, 2]])
                pm1 = _ap(PMO[:], 385, [[512, 128], [1, 1]])
                pm0 = _ap(PMO[:], 384, [[512, 128], [1, 1]])
                if w < 128:
                    nc.tensor.matmul(pm2, SD(w), vt[:, 0:2], start=True, stop=False, skip_group_check=True)
                    nc.tensor.matmul(pm1, SU(128 - w), vt[:, 0:1], start=False, stop=True, skip_group_check=True)
                else:
                    nc.tensor.matmul(pm0, SD(w - 128), vt[:, 0:1], start=True, stop=True, skip_group_check=True)

            def out_maint_copy(w):
                if w < 128:
                    nc.scalar.copy(_ap(TGB[:], 1024 + w, [[2048, 128], [256, 2]]),
                                   _ap(PMO[:], 384, [[512, 128], [1, 2]]))
                else:
                    nc.scalar.copy(_ap(TGB[:], 1280 + w, [[2048, 128], [1, 1]]),
                                   _ap(PMO[:], 384, [[512, 128], [1, 1]]))

            def out_bulk_A(u):
                # R1(u): merged 2-block matmuls: cols {1..L | 258..257+L} from
                # TGB {T-part | GU-part}; R2(u) same for g1 targets.
                L = 255 - u
                Rn = getR(u)
                oM = _ap(Rn[:], 1, [[512, 128], [257, 2], [1, L]])
                if u < 128:
                    nc.tensor.matmul(oM, SU(u), _ap(TGB[:], 1, [[2048, 128], [1025 + u, 2], [1, L]]),
                                     start=False, stop=False, skip_group_check=True)
                    nc.tensor.matmul(oM, SD(128 - u), _ap(TGB[:], 257, [[2048, 128], [1025 + u, 2], [1, L]]),
                                     start=False, stop=True, skip_group_check=True)
                    Rn2 = getR2(u)
                    pM = _ap(Rn2[:], 1, [[512, 128], [257, 2], [1, L]])
                    nc.tensor.matmul(pM, SU(u), _ap(TGB[:], 257, [[2048, 128], [1025 + u, 2], [1, L]]),
                                     start=False, stop=True, skip_group_check=True)
                else:
                    nc.tensor.matmul(oM, SU(u - 128), _ap(TGB[:], 257, [[2048, 128], [1025 + u, 2], [1, L]]),
                                     start=False, stop=True, skip_group_check=True)

            def out_stt(w):
                c0 = min(128, 256 - w)
                c1 = max(0, 128 - w)
                L = 255 - w
                R = getR(w)
                nc.vector.scalar_tensor_tensor(
                    out=_ap(SCR[:], 0, [[512, c0], [256, 2], [1, L]]),
                    in0=_ap(GTU[:], w + 1, [[1024, c0], [256 - w, 2], [1, L]]),
                    scalar=E[0:c0, w:w + 1],
                    in1=_ap(R[:], 1, [[512, c0], [256, 2], [1, L]]),
                    op0=MULT, op1=MULT,
                    accum_out=_ap(GTU[:], w, [[1024, c0], [1, 1]]))
                del Rt[w]
                if c1 > 0:
                    R2 = getR2(w)
                    nc.vector.scalar_tensor_tensor(
                        out=_ap(SCR[:], 0, [[512, c1], [256, 2], [1, L]]),
                        in0=_ap(GTU[:], 512 + w + 1, [[1024, c1], [256 - w, 2], [1, L]]),
                        scalar=E[0:c1, 256 + w:257 + w],
                        in1=_ap(R2[:], 1, [[512, c1], [256, 2], [1, L]]),
                        op0=MULT, op1=MULT,
                        accum_out=_ap(GTU[:], 512 + w, [[1024, c1], [1, 1]]))
                    del R2t[w]

            # ---------------- fused schedule ----------------
            LAG = 3

            nc.scalar.mul(T[:, 1:2], E[:, 1:2], float(np.exp(-C)))
            nc.scalar.mul(T[0:127, 257:258], E[0:127, 257:258], float(np.exp(-C)))
            vtl1 = split_in(1)
            in_chain_C(1, vtl1)
            in_maint_mm(1)
            in_maint_copy(1)

            def inside_pre(w):
                in_stt(w)
                if w == 255:
                    return None, False
                vtl = split_in(w)
                in_chain_C(w, vtl)
                in_maint_mm(w)
                return vtl, True

            def outside_pre(w):
                out_stt(w)
                if w == 1:
                    return None, False
                vto = split_out(w)
                out_chain_C(w, vto)
                out_maint_mm(w, vto)
                if w - 1 >= 1:
                    out_bulk_A(w - 1)
                return vto, True

            def outside_prologue():
                nc.scalar.copy(GTU[0:1, 255:256], E[0:1, 255:256])
                vto = split_out(255)
                out_chain_C(255, vto)
                out_bulk_A(254)
                out_maint_mm(255, vto)
                out_maint_copy(255)

            for tau in range(2, 258):
                w_in = tau if tau <= 255 else None
                u = 258 - tau if tau > LAG else None
                if u is not None and not (1 <= u <= 254):
                    u = None
                act_in = act_out = False
                if w_in is not None:
                    _, act_in = inside_pre(w_in)
                if tau == LAG:
                    outside_prologue()
                if u is not None:
                    _, act_out = outside_pre(u)
                # scalar copies: GU first (deps on pm-out), then U32+UB (pm-in)
                if act_out:
                    out_maint_copy(u)
                if act_in:
                    in_maint_copy(w_in)
                # inside bulk for R(w+1): chain(w) opened it this step; reads
                # UB col w-1 copied last step, so it never stalls the PE queue
                if act_in and w_in + 1 <= 255:
                    in_bulk_A(w_in + 1)

            if DEBUG:
                nc.sync.dma_start(out=tdump[:], in_=T[:])
                nc.sync.dma_start(out=gdump[:], in_=GTU[:])

            # ---------------- epilogue ----------------
            nc.vector.reciprocal(MISC[0:1, 0:1], T[0:1, 255:256])
            nc.vector.memset(ONES[:], 1.0)
            pb = _ap(PMI[:], 400, [[512, 128], [1, 1]])
            nc.tensor.matmul(pb, ONES[0:1, 0:128], MISC[0:1, 0:1], start=True, stop=True,
                             skip_group_check=True)
            nc.scalar.copy(MISC[:, 2:3], pb)
            nc.vector.tensor_tensor(out=SCR[:], in0=T[:],
                                    in1=_ap(GTU[:], 0, [[1024, 128], [512, 2], [1, 256]]), op=MULT)
            nc.vector.tensor_tensor(out=SCR[:], in0=SCR[:], in1=En[:], op=MULT)
            nc.vector.tensor_scalar_mul(SCR[:], SCR[:], MISC[:, 2:3])
            nc.sync.dma_start(out=_ap(out[:], 0, [[257, 128], [1, 256]]), in_=SCR[:, 0:256])
            nc.sync.dma_start(out=_ap(out[:], 128 * 257, [[257, 127], [1, 256]]), in_=SCR[0:127, 256:512])
    nc.compile()
    return nc


def _get_nc():
    if "nc" not in _cache:
        _cache["nc"] = build_nc()
        import ml_dtypes
        wu = np.zeros((128, 256), np.float32)
        wd = np.zeros((128, 384), np.float32)
        for k in range(128):
            wu[k, k] = 1.0
            wd[k, k + 128] = 1.0
        _cache["wu"] = np.asarray(wu.astype(ml_dtypes.bfloat16))
        _cache["wd"] = np.asarray(wd.astype(ml_dtypes.bfloat16))
    return _cache["nc"], _cache["wu"], _cache["wd"]


def kernel(scores: np.ndarray, mask: np.ndarray = None, **_unused) -> np.ndarray:
    nc, wu, wd = _get_nc()
    B = scores.shape[0]
    in_maps = [{"scores": np.ascontiguousarray(scores[b], dtype=np.float32),
                "wu": wu, "wd": wd} for b in range(B)]
    res = run_bass_kernel_spmd(nc, in_maps, list(range(B)))
    _cache["last_results"] = res
    return np.stack([res.results[b]["out"] for b in range(B)]).astype(np.float32)


# revision 18
# speedup vs baseline: 1.0665x; 1.0665x over previous
"""CRF constituency marginals (inside-outside, scaled exp space) on 8 TRN2 cores.

v5: inside and outside passes run CONCURRENTLY (outside width u needs only
inside diagonals <= 255-u; with a 3-step lag both 255-step serial chains
overlap, halving effective serial depth 509 -> 255 steps). Bulk R-alignment
uses bf16 hi-only chart values (lo residuals dropped; ~9e-3 rel err, under
the 2e-2 gate) and merged 2-block matmuls. Each R bank's accumulation group
is opened by the chain matmul (start=True) and closed by the same-step bulk,
so every PE op's inputs are >= 1 step old and nothing head-of-line blocks.
U/GU maintenance stages through two private PSUM banks (PMI/PMO) whose
reuse is serialized by WAR dependencies on fixed columns.

PSUM banks (8): psI x2 inside R | psR x2 outside R1 | psR2 x2 outside R2
 | psM x2 maintenance staging (PMI, PMO).
Charts are col-grouped: logical row r -> (partition r%128, col-block r//128).
"""
import sys, os
import numpy as np

for p in ("/opt/trn_rl_repo", "/root/.axon_site/_ro/trn_rl_repo"):
    if os.path.isdir(p) and p not in sys.path:
        sys.path.append(p)

import concourse.bass as bass
import concourse.bacc as bacc
import concourse.mybir as mybir
import concourse.tile as tile
from concourse.bass_utils import run_bass_kernel_spmd
import bass_rust

S = 256
C = 1.9
dt = mybir.dt.float32
bf = mybir.dt.bfloat16
MULT = mybir.AluOpType.mult
SUB = mybir.AluOpType.subtract
ADD = mybir.AluOpType.add
Exp = mybir.ActivationFunctionType.Exp

_cache = {}


def _ap(t_ap, offset, dims):
    return bass_rust.AP(tensor=t_ap.tensor, offset=int(offset),
                        ap=[[int(a), int(b)] for a, b in dims])


def build_nc():
    nc = bacc.Bacc("TRN2", target_bir_lowering=False)
    sc = nc.dram_tensor("scores", [S, S], dt, kind="ExternalInput")
    wu_d = nc.dram_tensor("wu", [128, 256], bf, kind="ExternalInput")
    wd_d = nc.dram_tensor("wd", [128, 384], bf, kind="ExternalInput")
    out = nc.dram_tensor("out", [S, S], dt, kind="ExternalOutput")
    DEBUG = bool(int(os.environ.get("K3_DEBUG", "0")))
    if DEBUG:
        tdump = nc.dram_tensor("tdump", [128, 512], dt, kind="ExternalOutput")
        gdump = nc.dram_tensor("gdump", [128, 1024], dt, kind="ExternalOutput")

    with tile.TileContext(nc) as tc:
        with tc.tile_pool(name="pers", bufs=1) as pool, \
             tc.tile_pool(name="vtp", bufs=4) as vtp, \
             tc.tile_pool(name="psI", bufs=2, space="PSUM") as psI, \
             tc.tile_pool(name="psR", bufs=2, space="PSUM") as psR, \
             tc.tile_pool(name="psR2", bufs=2, space="PSUM") as psR2, \
             tc.tile_pool(name="psM", bufs=1, space="PSUM") as psM:
            T = pool.tile([128, 512], dt, tag="T")       # inside chart by-start [g0|g1]
            GTU = pool.tile([128, 1024], dt, tag="GTU")  # [GTg0|U32g0|GTg1|U32g1]
            UB = pool.tile([128, 1024], bf, tag="UB")    # U hi [g0|g1|dump|dump]
            TGB = pool.tile([128, 2048], bf, tag="TGB")  # [Tg0hi|Tg1hi|-|-|GUg0hi|GUg1hi|-|-]
            E = pool.tile([128, 512], dt, tag="E")
            En = pool.tile([128, 512], dt, tag="En")
            SCR = pool.tile([128, 512], dt, tag="SCR")
            WU = pool.tile([128, 256], bf, tag="WU")
            WD = pool.tile([128, 384], bf, tag="WD")
            ONES = pool.tile([128, 128], dt, tag="ONES")
            MISC = pool.tile([128, 8], dt, tag="MISC")
            PMI = psM.tile([128, 512], dt, tag="PMI")    # inside maint staging
            PMO = psM.tile([128, 512], dt, tag="PMO")    # outside maint staging

            SU = lambda s: WU[:, s:s + 128]              # noqa: E731  out[p]=in[p+s]
            SD = lambda s: WD[:, 128 - s:256 - s]        # noqa: E731  out[p]=in[p-s]

            for t in (T, GTU, SCR):
                nc.vector.memset(t[:], 0.0)
            nc.vector.memset(UB[:], 0.0)
            nc.vector.memset(TGB[:], 0.0)
            nc.sync.dma_start(out=WU[:], in_=wu_d[:])
            nc.sync.dma_start(out=WD[:], in_=wd_d[:])
            # skewed score load: SCR[p, g*256+x] = scores[r, r+x], r = p+128g
            nc.sync.dma_start(out=SCR[:, 0:256], in_=_ap(sc[:], 0, [[257, 128], [1, 256]]))
            nc.sync.dma_start(out=SCR[0:127, 256:512], in_=_ap(sc[:], 128 * 257, [[257, 127], [1, 256]]))
            nc.scalar.activation(E[:], SCR[:], Exp, bias=0.0, scale=1.0)
            nc.scalar.activation(En[:], SCR[:], Exp, bias=0.0, scale=-1.0)

            Rit = {}

            def getRI(w):
                if w not in Rit:
                    Rit[w] = psI.tile([128, 512], dt, tag="RI", name=f"RI{w}")
                return Rit[w]

            Rt = {}

            def getR(w):
                if w not in Rt:
                    Rt[w] = psR.tile([128, 512], dt, tag="R", name=f"R{w}")
                return Rt[w]

            R2t = {}

            def getR2(w):
                if w not in R2t:
                    R2t[w] = psR2.tile([128, 512], dt, tag="R2", name=f"R2_{w}")
                return R2t[w]

            # ---------------- INSIDE ----------------
            def tgb_hi2(w):
                return _ap(TGB[:], w, [[2048, 128], [256, 2]])

            def split_in(w):
                # hi (bf16) of T diag w -> TGB cols {w, 256+w}; lo -> vtl when needed
                src = _ap(T[:], w, [[512, 128], [256, 2]])
                nc.vector.tensor_copy(tgb_hi2(w), src)
                if w < 16:
                    vtl = vtp.tile([128, 8], bf, tag="vtl", name=f"vtl{w}")
                    nc.vector.tensor_tensor(out=vtl[:, 0:2], in0=src, in1=tgb_hi2(w), op=SUB)
                    return vtl
                return None

            def in_chain_C(w, vtl):
                # R(w+1) col m=1 = SU_1(val_w); boundary via SD_127 when val has g1 rows
                Rn = getRI(w + 1)
                lo = (w < 16)
                hi2 = tgb_hi2(w)
                hi_g0 = _ap(TGB[:], w, [[2048, 128], [1, 1]])
                hi_g1 = _ap(TGB[:], 256 + w, [[2048, 128], [1, 1]])
                ops = []
                if w + 1 <= 127:
                    o = _ap(Rn[:], 1, [[512, 128], [256, 2], [1, 1]])
                    ops.append((o, SU(1), hi2))
                    if lo:
                        ops.append((o, SU(1), vtl[:, 0:2]))
                else:
                    o = _ap(Rn[:], 1, [[512, 128], [1, 1]])
                    ops.append((o, SU(1), hi_g0))
                    if lo:
                        ops.append((o, SU(1), vtl[:, 0:1]))
                if w < 128:
                    o0 = _ap(Rn[:], 1, [[512, 128], [1, 1]])
                    ops.append((o0, SD(127), hi_g1))
                    if lo:
                        ops.append((o0, SD(127), vtl[:, 1:2]))
                for i, (oo, ss, mm) in enumerate(ops):
                    nc.tensor.matmul(oo, ss, mm, start=(i == 0),
                                     stop=(w == 1 and i == len(ops) - 1),
                                     skip_group_check=True)

            def in_maint_mm(w):
                # U col w (shift t(w) by w) staged in PM cols {384,385}
                hi2 = tgb_hi2(w)
                hi_g0 = _ap(TGB[:], w, [[2048, 128], [1, 1]])
                pm2 = _ap(PMI[:], 384, [[512, 128], [1, 2]])
                pm1 = _ap(PMI[:], 385, [[512, 128], [1, 1]])
                pm0 = _ap(PMI[:], 384, [[512, 128], [1, 1]])
                if w < 128:
                    nc.tensor.matmul(pm2, SD(w), hi2, start=True, stop=False, skip_group_check=True)
                    nc.tensor.matmul(pm1, SU(128 - w), hi_g0, start=False, stop=True, skip_group_check=True)
                else:
                    nc.tensor.matmul(pm0, SD(w - 128), hi_g0, start=True, stop=True, skip_group_check=True)

            def in_maint_copy(w):
                # U32 <- PM (fp32), then UB <- PM (bf16) with dummy read of the
                # pm-out cols {388,389} so later PE writes to PM wait for it.
                if w < 128:
                    nc.scalar.copy(_ap(GTU[:], 256 + w, [[1024, 128], [512, 2]]),
                                   _ap(PMI[:], 384, [[512, 128], [1, 2]]))
                    nc.scalar.copy(_ap(UB[:], w, [[1024, 128], [256, 2]]),
                                   _ap(PMI[:], 384, [[512, 128], [1, 2]]))
                else:
                    nc.scalar.copy(_ap(GTU[:], 768 + w, [[1024, 128], [1, 1]]),
                                   _ap(PMI[:], 384, [[512, 128], [1, 1]]))
                    nc.scalar.copy(_ap(UB[:], 256 + w, [[1024, 128], [1, 1]]),
                                   _ap(PMI[:], 384, [[512, 128], [1, 1]]))

            def in_bulk_A(wt):
                # R(wt) cols m=2..wt-1 <- U cols u=wt-m (rev), via SU/SD bands
                n = wt - 2
                if n <= 0:
                    return
                Rn = getRI(wt)
                u0 = wt - 2  # newest U col read
                if wt <= 127:
                    o = _ap(Rn[:], 2, [[512, 128], [256, 2], [1, n]])
                    nc.tensor.matmul(o, SU(wt), _ap(UB[:], u0, [[1024, 128], [256, 2], [-1, n]]),
                                     start=False, stop=False, skip_group_check=True)
                    o0 = _ap(Rn[:], 2, [[512, 128], [1, n]])
                    nc.tensor.matmul(o0, SD(128 - wt), _ap(UB[:], 256 + u0, [[1024, 128], [-1, n]]),
                                     start=False, stop=True, skip_group_check=True)
                else:
                    o0 = _ap(Rn[:], 2, [[512, 128], [1, n]])
                    nc.tensor.matmul(o0, SU(wt - 128), _ap(UB[:], 256 + u0, [[1024, 128], [-1, n]]),
                                     start=False, stop=True, skip_group_check=True)

            def in_stt(w):
                c0 = min(128, 256 - w)
                c1 = max(0, 128 - w)
                R = getRI(w)
                nc.vector.scalar_tensor_tensor(
                    out=SCR[0:c0, 0:w - 1], in0=T[0:c0, 1:w], scalar=E[0:c0, w:w + 1],
                    in1=_ap(R[:], 1, [[512, c0], [1, w - 1]]), op0=MULT, op1=MULT,
                    accum_out=T[0:c0, w:w + 1])
                if c1 > 0:
                    nc.vector.scalar_tensor_tensor(
                        out=SCR[0:c1, 256:256 + w - 1], in0=T[0:c1, 257:256 + w],
                        scalar=E[0:c1, 256 + w:257 + w],
                        in1=_ap(R[:], 257, [[512, c1], [1, w - 1]]), op0=MULT, op1=MULT,
                        accum_out=T[0:c1, 256 + w:257 + w])
                del Rit[w]

            # ---------------- OUTSIDE ----------------
            def split_out(w):
                vt = vtp.tile([128, 8], bf, tag="vt", name=f"vto{w}")
                src = _ap(GTU[:], w, [[1024, 128], [512, 2]])
                nc.vector.tensor_copy(vt[:, 0:2], src)
                if w >= 240:
                    nc.vector.tensor_tensor(out=vt[:, 2:4], in0=src, in1=vt[:, 0:2], op=SUB)
                return vt

            def out_chain_C(w, vt):
                # R1(w-1) col s=1 = SD_1(gval_w); g1 gets SD_1(valg1) + SU_127(valg0)
                Rn = getR(w - 1)
                lo = (w >= 240)
                o = _ap(Rn[:], 257, [[512, 128], [1, 1]])
                if lo:
                    nc.tensor.matmul(o, SD(1), vt[:, 0:1], start=True, stop=False, skip_group_check=True)
                    nc.tensor.matmul(o, SD(1), vt[:, 2:3], start=False, stop=False, skip_group_check=True)
                else:
                    nc.tensor.matmul(o, SD(1), vt[:, 0:1], start=True, stop=False, skip_group_check=True)
                if w <= 128:
                    Rn2 = getR2(w - 1)
                    o2 = _ap(Rn2[:], 257, [[512, 128], [1, 1]])
                    nc.tensor.matmul(o2, SD(1), vt[:, 1:2], start=True, stop=False, skip_group_check=True)
                    nc.tensor.matmul(o2, SU(127), vt[:, 0:1], start=False, stop=False, skip_group_check=True)

            def out_maint_mm(w, vt):
                # GU col w staged in PM cols {388,389}
                pm2 = _ap(PMO[:], 384, [[512, 128], [1, 2]])
                pm1 = _ap(PMO[:], 385, [[512, 128], [1, 1]])
                pm0 = _ap(PMO[:], 384, [[512, 128], [1, 1]])
                if w < 128:
                    nc.tensor.matmul(pm2, SD(w), vt[:, 0:2], start=True, stop=False, skip_group_check=True)
                    nc.tensor.matmul(pm1, SU(128 - w), vt[:, 0:1], start=False, stop=True, skip_group_check=True)
                else:
                    nc.tensor.matmul(pm0, SD(w - 128), vt[:, 0:1], start=True, stop=True, skip_group_check=True)

            def out_maint_copy(w):
                if w < 128:
                    nc.scalar.copy(_ap(TGB[:], 1024 + w, [[2048, 128], [256, 2]]),
                                   _ap(PMO[:], 384, [[512, 128], [1, 2]]))
                else:
                    nc.scalar.copy(_ap(TGB[:], 1280 + w, [[2048, 128], [1, 1]]),
                                   _ap(PMO[:], 384, [[512, 128], [1, 1]]))

            def out_bulk_A(u):
                # R1(u): merged 2-block matmuls: cols {1..L | 258..257+L} from
                # TGB {T-part | GU-part}; R2(u) same for g1 targets.
                L = 255 - u
                Rn = getR(u)
                oM = _ap(Rn[:], 1, [[512, 128], [257, 2], [1, L]])
                if u < 128:
                    nc.tensor.matmul(oM, SU(u), _ap(TGB[:], 1, [[2048, 128], [1025 + u, 2], [1, L]]),
                                     start=False, stop=False, skip_group_check=True)
                    nc.tensor.matmul(oM, SD(128 - u), _ap(TGB[:], 257, [[2048, 128], [1025 + u, 2], [1, L]]),
                                     start=False, stop=True, skip_group_check=True)
                    Rn2 = getR2(u)
                    pM = _ap(Rn2[:], 1, [[512, 128], [257, 2], [1, L]])
                    nc.tensor.matmul(pM, SU(u), _ap(TGB[:], 257, [[2048, 128], [1025 + u, 2], [1, L]]),
                                     start=False, stop=True, skip_group_check=True)
                else:
                    nc.tensor.matmul(oM, SU(u - 128), _ap(TGB[:], 257, [[2048, 128], [1025 + u, 2], [1, L]]),
                                     start=False, stop=True, skip_group_check=True)

            def out_stt(w):
                c0 = min(128, 256 - w)
                c1 = max(0, 128 - w)
                L = 255 - w
                R = getR(w)
                nc.vector.scalar_tensor_tensor(
                    out=_ap(SCR[:], 0, [[512, c0], [256, 2], [1, L]]),
                    in0=_ap(GTU[:], w + 1, [[1024, c0], [256 - w, 2], [1, L]]),
                    scalar=E[0:c0, w:w + 1],
                    in1=_ap(R[:], 1, [[512, c0], [256, 2], [1, L]]),
                    op0=MULT, op1=MULT,
                    accum_out=_ap(GTU[:], w, [[1024, c0], [1, 1]]))
                del Rt[w]
                if c1 > 0:
                    R2 = getR2(w)
                    nc.vector.scalar_tensor_tensor(
                        out=_ap(SCR[:], 0, [[512, c1], [256, 2], [1, L]]),
                        in0=_ap(GTU[:], 512 + w + 1, [[1024, c1], [256 - w, 2], [1, L]]),
                        scalar=E[0:c1, 256 + w:257 + w],
                        in1=_ap(R2[:], 1, [[512, c1], [256, 2], [1, L]]),
                        op0=MULT, op1=MULT,
                        accum_out=_ap(GTU[:], 512 + w, [[1024, c1], [1, 1]]))
                    del R2t[w]

            # ---------------- fused schedule ----------------
            LAG = 3

            nc.scalar.mul(T[:, 1:2], E[:, 1:2], float(np.exp(-C)))
            nc.scalar.mul(T[0:127, 257:258], E[0:127, 257:258], float(np.exp(-C)))
            vtl1 = split_in(1)
            in_chain_C(1, vtl1)
            in_maint_mm(1)
            in_maint_copy(1)

            def inside_pre(w):
                in_stt(w)
                if w == 255:
                    return None, False
                vtl = split_in(w)
                in_chain_C(w, vtl)
                in_maint_mm(w)
                return vtl, True

            def outside_pre(w):
                out_stt(w)
                if w == 1:
                    return None, False
                vto = split_out(w)
                out_chain_C(w, vto)
                out_maint_mm(w, vto)
                if w - 1 >= 1:
                    out_bulk_A(w - 1)
                return vto, True

            def outside_prologue():
                nc.scalar.copy(GTU[0:1, 255:256], E[0:1, 255:256])
                vto = split_out(255)
                out_chain_C(255, vto)
                out_bulk_A(254)
                out_maint_mm(255, vto)
                out_maint_copy(255)

            for tau in range(2, 258):
                w_in = tau if tau <= 255 else None
                u = 258 - tau if tau > LAG else None
                if u is not None and not (1 <= u <= 254):
                    u = None
                act_in = act_out = False
                if w_in is not None:
                    _, act_in = inside_pre(w_in)
                if tau == LAG:
                    outside_prologue()
                if u is not None:
                    _, act_out = outside_pre(u)
                # scalar copies: GU first (deps on pm-out), then U32+UB (pm-in)
                if act_out:
                    out_maint_copy(u)
                if act_in:
                    in_maint_copy(w_in)
                # inside bulk for R(w+1): chain(w) opened it this step; reads
                # UB col w-1 copied last step, so it never stalls the PE queue
                if act_in and w_in + 1 <= 255:
                    in_bulk_A(w_in + 1)

            if DEBUG:
                nc.sync.dma_start(out=tdump[:], in_=T[:])
                nc.sync.dma_start(out=gdump[:], in_=GTU[:])

            # ---------------- epilogue ----------------
            nc.vector.reciprocal(MISC[0:1, 0:1], T[0:1, 255:256])
            nc.vector.memset(ONES[:], 1.0)
            pb = _ap(PMI[:], 400, [[512, 128], [1, 1]])
            nc.tensor.matmul(pb, ONES[0:1, 0:128], MISC[0:1, 0:1], start=True, stop=True,
                             skip_group_check=True)
            nc.scalar.copy(MISC[:, 2:3], pb)
            nc.vector.tensor_tensor(out=SCR[:], in0=T[:],
                                    in1=_ap(GTU[:], 0, [[1024, 128], [512, 2], [1, 256]]), op=MULT)
            nc.vector.tensor_tensor(out=SCR[:], in0=SCR[:], in1=En[:], op=MULT)
            nc.vector.tensor_scalar_mul(SCR[:], SCR[:], MISC[:, 2:3])
            nc.sync.dma_start(out=_ap(out[:], 0, [[257, 128], [1, 256]]), in_=SCR[:, 0:256])
            nc.sync.dma_start(out=_ap(out[:], 128 * 257, [[257, 127], [1, 256]]), in_=SCR[0:127, 256:512])
    nc.compile()
    return nc


def _get_nc():
    if "nc" not in _cache:
        _cache["nc"] = build_nc()
        import ml_dtypes
        wu = np.zeros((128, 256), np.float32)
        wd = np.zeros((128, 384), np.float32)
        for k in range(128):
            wu[k, k] = 1.0
            wd[k, k + 128] = 1.0
        _cache["wu"] = np.asarray(wu.astype(ml_dtypes.bfloat16))
        _cache["wd"] = np.asarray(wd.astype(ml_dtypes.bfloat16))
    return _cache["nc"], _cache["wu"], _cache["wd"]


def kernel(scores: np.ndarray, mask: np.ndarray = None, **_unused) -> np.ndarray:
    nc, wu, wd = _get_nc()
    B = scores.shape[0]
    in_maps = [{"scores": np.ascontiguousarray(scores[b], dtype=np.float32),
                "wu": wu, "wd": wd} for b in range(B)]
    res = run_bass_kernel_spmd(nc, in_maps, list(range(B)))
    _cache["last_results"] = res
    return np.stack([res.results[b]["out"] for b in range(B)]).astype(np.float32)
